# revision 34
# baseline (speedup 1.0000x reference)
"""BertAdapter (TT-decomposed bottleneck MLP) Trainium2 kernel.

Computes  out = x + gelu(x @ W_down + b_down) @ W_up + b_up  where the
adapter weights arrive as tensor-train cores.  The TT cores are tiny
(~50K params), so they are contracted to dense matrices on the host and
the device kernel runs the dense bottleneck MLP data-parallel across
8 NeuronCores (2 batches of 2048 tokens per core).

The kernel is HBM-bandwidth dominated (25 MB of fp32 I/O per core vs
~20 us of PE work), so all HBM traffic is bf16 (halves the DMA floor
to ~33 us; rel-err ~5e-3 against the 2e-2 gate since PSUM accumulation
stays fp32).

The whole device pipeline runs in the TRANSPOSED domain: the host hands
each core x^T (hidden-major) and receives y^T back.  hidden-on-
partitions is exactly what the PE needs for the down-projection, so the
kernel has NO on-device transposes (an earlier natural-layout version
spent ~15 us/body of PE time on transposes plus a PSUM->SBUF drain on
the scalar engine just to feed the same matmuls).

Per-core device kernel, per 512-token block:
  1. One DMA brings in x^T [768, 512] as [128, 6, 512] bf16
     (hidden on partitions), issued from the SP sequencer (pure input
     prefetch stream; outputs leave via GPSIMD SWDGE).
  2. Down-proj: 6 accumulating bf16 matmuls -> PSUM [65, 512].
  3. Exact-erf Gelu + b_down bias on the scalar engine (bias is
     per-partition in this layout); row 64 evaluates
     gelu(gelu^-1(1)) = 1, the ones-row that multiplies the b_up row
     folded into wub.
  4. Up-proj, transposed: per hidden chunk, W_up chunk [65, 128] is the
     stationary operand and act [65, 512] moves -> PSUM [128, 512].
     PSUM pool is 6 deep, so the PE stream never waits on the drains.
  5. Residual add (x^T + up^T) on the vector engine -> bf16; chunks 4-5
     are routed ACT-copy + cheap 2x SBUF add to keep DVE off the
     critical path.  One DMA out (GPSIMD).
"""

import os
import sys
from contextlib import ExitStack

import numpy as np
import ml_dtypes

for _p in ("/opt/trn_rl_repo", "/root/.axon_site/_ro/trn_rl_repo"):
    if os.path.isdir(_p) and _p not in sys.path:
        sys.path.insert(0, _p)

import concourse.bass as bass
import concourse.tile as tile
from concourse import mybir
from concourse.bass_utils import run_bass_kernel_spmd

P = 128                 # SBUF partitions
H = 768                 # hidden size
A = 64                  # adapter bottleneck size
B, S = 16, 2048         # full batch / seq
NCORES = 8
TOK = (B // NCORES) * S  # tokens per core = 4096
TBLK = 512              # tokens per pipeline block
NBLK = TOK // TBLK
HC = H // P             # hidden chunks of 128
F32 = mybir.dt.float32
BF16 = mybir.dt.bfloat16
NPBF16 = ml_dtypes.bfloat16
X_SHAPE = (NBLK, P, HC, TBLK)  # per-core transposed + blocked device I/O


_TileContext = tile.TileContext


def _legalize_waits(nc):
    """Split multi-wait instructions for this walrus build.

    The walrus in this toolchain accepts only ONE sync-wait per
    instruction ("Too many sync wait commands" in setupSyncWait), while
    Tile freely attaches several.  Hoist all but the last wait of each
    instruction onto freshly inserted same-engine NoOps directly before
    it — engine program order makes this semantically identical.
    """
    n = 0

    def fix_block(bb):
        nonlocal n
        insts = bb.instructions
        i = 0
        while i < len(insts):
            inst = insts[i]
            for sub in getattr(inst, "blocks", None) or []:
                fix_block(sub)
            si = inst.sync_info
            waits = list(si.on_wait) if si and si.on_wait else []
            if len(waits) > 1:
                for w in waits[:-1]:
                    nop = mybir.InstNoOp(name=f"I-waitsplit-{n}", ins=[], outs=[])
                    n += 1
                    nop.engine = inst.engine
                    nop.sync_info = mybir.SyncInfo(on_wait=[w], on_update=[])
                    insts.insert(i, nop)
                    i += 1
                inst.sync_info = mybir.SyncInfo(
                    on_wait=[waits[-1]], on_update=list(si.on_update)
                )
            i += 1

    for fn in nc.m.functions:
        for bb in fn.blocks:
            fix_block(bb)
    return nc


def build_nc(tok=TOK, repeats=1, mode="full"):
    nblk = tok // TBLK
    nc = bass.Bass("TRN2", target_bir_lowering=False, debug=False)
    # x/y are the per-core shard transposed AND pre-blocked on the host to
    # [block, partition, chunk, token]: every per-block DMA is one fully
    # contiguous 786 KB read/write with 6 KB per partition line
    x = nc.dram_tensor(
        "x", [nblk, P, HC, TBLK], BF16, kind="ExternalInput"
    ).ap()
    # wd/bd carry an extra adapter column: wd col A is zeros and bd[A] is
    # gelu^-1(1.0), so the gelu writes a constant ones-row into act[A] that
    # multiplies the b_up row of wub in the up-projection (bias via matmul).
    wd = nc.dram_tensor("wd", [H, A + 1], BF16, kind="ExternalInput").ap()
    wub = nc.dram_tensor("wub", [A + 1, H], BF16, kind="ExternalInput").ap()
    bd = nc.dram_tensor("bd", [A + 1, 1], F32, kind="ExternalInput").ap()
    y = nc.dram_tensor(
        "y", [nblk, P, HC, TBLK], BF16, kind="ExternalOutput"
    ).ap()

    with ExitStack() as ctx:
        tc = ctx.enter_context(_TileContext(nc))
        const = ctx.enter_context(tc.tile_pool(name="const", bufs=1))
        xin = ctx.enter_context(tc.tile_pool(name="xin", bufs=3))
        actp = ctx.enter_context(tc.tile_pool(name="act", bufs=2))
        outp = ctx.enter_context(tc.tile_pool(name="out", bufs=2))
        ubp = ctx.enter_context(tc.tile_pool(name="ub", bufs=3))
        ps_d = ctx.enter_context(tc.tile_pool(name="ps_d", bufs=2, space="PSUM"))
        ps_u = ctx.enter_context(tc.tile_pool(name="ps_u", bufs=6, space="PSUM"))

        wd_sb = const.tile([P, HC, A + 1], BF16)
        nc.sync.dma_start(wd_sb[:], wd.rearrange("(c p) a -> p c a", p=P))
        wub_sb = const.tile([A + 1, H], BF16)
        nc.sync.dma_start(wub_sb[:], wub[:])
        bd_sb = const.tile([A + 1, 1], F32)
        nc.sync.dma_start(bd_sb[:], bd[:])
        # touch the Gelu table set up front so its ACT_TABLE_LOAD overlaps
        # the first input DMA instead of stalling the first block
        warm = const.tile([1, 1], F32)
        nc.scalar.activation(
            warm[:], bd_sb[0:1, 0:1], mybir.ActivationFunctionType.Gelu
        )

        # batch four 786 KB blocks per DMA: 3.14 MB transfers amortize the
        # fixed trigger + completion-receipt cost (~2 us each on SWDGE)
        # over 2x the bytes and sit higher on the SDMA efficiency curve;
        # the descriptor profile (6 KB contiguous per partition line) is
        # unchanged, as is all compute/PSUM structure
        x_pair = x.rearrange("(u k) p c t -> u p k c t", k=4)
        y_pair = y.rearrange("(u k) p c t -> u p k c t", k=4)
        npair = nblk // 4

        for u in range(npair * repeats):
            u = u % npair
            xt2 = xin.tile([P, 4, HC, TBLK], BF16, tag="xin")
            nc.sync.dma_start(xt2[:], x_pair[u])
            if mode == "dmaonly":
                ot2 = outp.tile([P, 4, HC, TBLK], BF16, tag="ot")
                nc.vector.tensor_copy(ot2[:, 0, 0, :], xt2[:, 0, 0, :])
                nc.gpsimd.dma_start(y_pair[u], ot2[:])
                continue
            ot2 = outp.tile([P, 4, HC, TBLK], BF16, tag="ot")
            for k in range(4):
                xt_sb = xt2[:, k]
                # down projection: accumulate over hidden chunks
                pd = ps_d.tile([A + 1, TBLK], F32)
                for c in range(HC):
                    nc.tensor.matmul(
                        pd[:],
                        wd_sb[:, c, :],
                        xt_sb[:, c, :],
                        start=(c == 0),
                        stop=(c == HC - 1),
                    )
                # exact-erf gelu with per-partition b_down bias; row A
                # computes gelu(0 + gelu^-1(1)) = 1.0, the b_up multiplier
                act = actp.tile([A + 1, TBLK], BF16)
                nc.scalar.activation(
                    act[:], pd[:], mybir.ActivationFunctionType.Gelu,
                    bias=bd_sb[:, 0:1],
                )
                if mode == "front":
                    continue
                # up projection, transposed: W_up chunk stationary, act
                # moving
                for c in range(HC):
                    pu = ps_u.tile([P, TBLK], F32)
                    nc.tensor.matmul(
                        pu[:], wub_sb[:, c * P : (c + 1) * P], act[:],
                        start=True, stop=True,
                    )
                    if mode != "dveonly" and c >= HC - 2:
                        # last two chunks: drain via ACT copy + 2x SBUF add
                        # so DVE (the busiest vector engine) stays under
                        # budget
                        ub = ubp.tile([P, TBLK], BF16)
                        nc.scalar.copy(ub[:], pu[:])
                        nc.vector.tensor_add(
                            ot2[:, k, c, :], xt_sb[:, c, :], ub[:]
                        )
                    else:
                        nc.vector.tensor_add(
                            ot2[:, k, c, :], xt_sb[:, c, :], pu[:]
                        )
            if mode not in ("noout", "front"):
                nc.gpsimd.dma_start(y_pair[u], ot2[:])
    return _legalize_waits(nc)


def _tt_to_matrix(cores, in_dim, out_dim):
    t = cores[0]
    for c in cores[1:]:
        t = np.tensordot(t, c, axes=([-1], [0]))
    t = np.squeeze(t, axis=(0, -1))
    return np.ascontiguousarray(t.reshape(in_dim, out_dim).astype(np.float32))


def _gelu_inv_one():
    """x with x * Phi(x) == 1 (erf gelu), solved by Newton in float64."""
    import math

    def gelu(x):
        return x * 0.5 * (1.0 + math.erf(x / math.sqrt(2.0)))

    def dgelu(x):
        return 0.5 * (1.0 + math.erf(x / math.sqrt(2.0))) + x * math.exp(
            -0.5 * x * x
        ) / math.sqrt(2.0 * math.pi)

    x = 1.15
    for _ in range(40):
        x -= (gelu(x) - 1.0) / dgelu(x)
    return x


_NC_CACHE = {}


def _get_nc(tok=TOK):
    if tok not in _NC_CACHE:
        _NC_CACHE[tok] = build_nc(tok)
    return _NC_CACHE[tok]


def kernel(hidden_states, d0, d1, d2, d3, d4, u0, u1, u2, u3, u4,
           b_down, b_up, **_run_kwargs):
    hs = np.asarray(hidden_states, dtype=np.float32)
    w_down = _tt_to_matrix(
        [np.asarray(c, np.float32) for c in (d0, d1, d2, d3, d4)], H, A
    )
    w_up = _tt_to_matrix(
        [np.asarray(c, np.float32) for c in (u0, u1, u2, u3, u4)], A, H
    )
    wd = np.concatenate([w_down, np.zeros((H, 1), np.float32)], axis=1)
    wd = np.ascontiguousarray(wd.astype(NPBF16))
    wub = np.ascontiguousarray(
        np.concatenate(
            [w_up, np.asarray(b_up, np.float32)[None, :]], axis=0
        ).astype(NPBF16)
    )
    bd = np.concatenate(
        [
            np.asarray(b_down, np.float32).reshape(A, 1),
            np.full((1, 1), _gelu_inv_one(), np.float32),
        ],
        axis=0,
    )
    bd = np.ascontiguousarray(bd)

    flat = hs.reshape(B * S, H).astype(NPBF16)

    def to_blocked(xc):
        # [4096, 768] -> [block, partition, chunk, token] with
        # token = b*TBLK + t, hidden = c*P + p
        return np.ascontiguousarray(
            xc.reshape(NBLK, TBLK, HC, P).transpose(0, 3, 2, 1)
        )

    in_maps = [
        {
            "x": to_blocked(flat[c * TOK : (c + 1) * TOK]),
            "wd": wd,
            "wub": wub,
            "bd": bd,
        }
        for c in range(NCORES)
    ]
    nc = _get_nc()
    res = run_bass_kernel_spmd(nc, in_maps, list(range(NCORES)), **_run_kwargs)
    out = np.concatenate(
        [
            res.results[c]["y"].transpose(0, 3, 2, 1).reshape(TOK, H)
            for c in range(NCORES)
        ],
        axis=0,
    )
    out = out.astype(np.float32).reshape(B, S, H)
    if _run_kwargs:
        kernel.last_results = res
    return out


# revision 36
# speedup vs baseline: 1.0427x; 1.0427x over previous
"""BertAdapter (TT-decomposed bottleneck MLP) Trainium2 kernel.

Computes  out = x + gelu(x @ W_down + b_down) @ W_up + b_up  where the
adapter weights arrive as tensor-train cores.  The TT cores are tiny
(~50K params), so they are contracted to dense matrices on the host and
the device kernel runs the dense bottleneck MLP data-parallel across
8 NeuronCores (2 batches of 2048 tokens per core).

The kernel is HBM-bandwidth dominated (25 MB of fp32 I/O per core vs
~20 us of PE work), so all HBM traffic is bf16 (halves the DMA floor
to ~33 us; rel-err ~5e-3 against the 2e-2 gate since PSUM accumulation
stays fp32).

The whole device pipeline runs in the TRANSPOSED domain: the host hands
each core x^T (hidden-major) and receives y^T back.  hidden-on-
partitions is exactly what the PE needs for the down-projection, so the
kernel has NO on-device transposes (an earlier natural-layout version
spent ~15 us/body of PE time on transposes plus a PSUM->SBUF drain on
the scalar engine just to feed the same matmuls).

Per-core device kernel, per 512-token block:
  1. One DMA brings in x^T [768, 512] as [128, 6, 512] bf16
     (hidden on partitions), issued from the SP sequencer (pure input
     prefetch stream; outputs leave via GPSIMD SWDGE).
  2. Down-proj: 6 accumulating bf16 matmuls -> PSUM [65, 512].
  3. Exact-erf Gelu + b_down bias on the scalar engine (bias is
     per-partition in this layout); row 64 evaluates
     gelu(gelu^-1(1)) = 1, the ones-row that multiplies the b_up row
     folded into wub.
  4. Up-proj, transposed: per hidden chunk, W_up chunk [65, 128] is the
     stationary operand and act [65, 512] moves -> PSUM [128, 512].
     PSUM pool is 6 deep, so the PE stream never waits on the drains.
  5. Residual add (x^T + up^T) on the vector engine -> bf16; chunks 4-5
     are routed ACT-copy + cheap 2x SBUF add to keep DVE off the
     critical path.  One DMA out (GPSIMD).
"""

import os
import sys
from contextlib import ExitStack

import numpy as np
import ml_dtypes

for _p in ("/opt/trn_rl_repo", "/root/.axon_site/_ro/trn_rl_repo"):
    if os.path.isdir(_p) and _p not in sys.path:
        sys.path.insert(0, _p)

import concourse.bass as bass
import concourse.tile as tile
from concourse import mybir
from concourse.bass_utils import run_bass_kernel_spmd

P = 128                 # SBUF partitions
H = 768                 # hidden size
A = 64                  # adapter bottleneck size
B, S = 16, 2048         # full batch / seq
NCORES = 8
TOK = (B // NCORES) * S  # tokens per core = 4096
TBLK = 512              # tokens per pipeline block
NBLK = TOK // TBLK
HC = H // P             # hidden chunks of 128
F32 = mybir.dt.float32
BF16 = mybir.dt.bfloat16
NPBF16 = ml_dtypes.bfloat16
X_SHAPE = (NBLK, P, HC, TBLK)  # per-core transposed + blocked device I/O


_TileContext = tile.TileContext


def _legalize_waits(nc):
    """Split multi-wait instructions for this walrus build.

    The walrus in this toolchain accepts only ONE sync-wait per
    instruction ("Too many sync wait commands" in setupSyncWait), while
    Tile freely attaches several.  Hoist all but the last wait of each
    instruction onto freshly inserted same-engine NoOps directly before
    it — engine program order makes this semantically identical.
    """
    n = 0

    def fix_block(bb):
        nonlocal n
        insts = bb.instructions
        i = 0
        while i < len(insts):
            inst = insts[i]
            for sub in getattr(inst, "blocks", None) or []:
                fix_block(sub)
            si = inst.sync_info
            waits = list(si.on_wait) if si and si.on_wait else []
            if len(waits) > 1:
                for w in waits[:-1]:
                    nop = mybir.InstNoOp(name=f"I-waitsplit-{n}", ins=[], outs=[])
                    n += 1
                    nop.engine = inst.engine
                    nop.sync_info = mybir.SyncInfo(on_wait=[w], on_update=[])
                    insts.insert(i, nop)
                    i += 1
                inst.sync_info = mybir.SyncInfo(
                    on_wait=[waits[-1]], on_update=list(si.on_update)
                )
            i += 1

    for fn in nc.m.functions:
        for bb in fn.blocks:
            fix_block(bb)
    return nc


def build_nc(tok=TOK, repeats=1, mode="full"):
    nblk = tok // TBLK
    nc = bass.Bass("TRN2", target_bir_lowering=False, debug=False)
    # x/y are the per-core shard transposed AND pre-blocked on the host to
    # [block, partition, chunk, token]: every per-block DMA is one fully
    # contiguous 786 KB read/write with 6 KB per partition line
    x = nc.dram_tensor(
        "x", [nblk, P, HC, TBLK], BF16, kind="ExternalInput"
    ).ap()
    # wd/bd carry an extra adapter column: wd col A is zeros and bd[A] is
    # gelu^-1(1.0), so the gelu writes a constant ones-row into act[A] that
    # multiplies the b_up row of wub in the up-projection (bias via matmul).
    wd = nc.dram_tensor("wd", [H, A + 1], BF16, kind="ExternalInput").ap()
    wub = nc.dram_tensor("wub", [A + 1, H], BF16, kind="ExternalInput").ap()
    bd = nc.dram_tensor("bd", [A + 1, 1], F32, kind="ExternalInput").ap()
    y = nc.dram_tensor(
        "y", [nblk, P, HC, TBLK], BF16, kind="ExternalOutput"
    ).ap()

    with ExitStack() as ctx:
        tc = ctx.enter_context(_TileContext(nc))
        const = ctx.enter_context(tc.tile_pool(name="const", bufs=1))
        xin = ctx.enter_context(tc.tile_pool(name="xin", bufs=2))
        actp = ctx.enter_context(tc.tile_pool(name="act", bufs=2))
        outp = ctx.enter_context(tc.tile_pool(name="out", bufs=4))
        ubp = ctx.enter_context(tc.tile_pool(name="ub", bufs=3))
        ps_d = ctx.enter_context(tc.tile_pool(name="ps_d", bufs=2, space="PSUM"))
        ps_u = ctx.enter_context(tc.tile_pool(name="ps_u", bufs=6, space="PSUM"))

        wd_sb = const.tile([P, HC, A + 1], BF16)
        nc.sync.dma_start(wd_sb[:], wd.rearrange("(c p) a -> p c a", p=P))
        wub_sb = const.tile([A + 1, H], BF16)
        nc.sync.dma_start(wub_sb[:], wub[:])
        bd_sb = const.tile([A + 1, 1], F32)
        nc.sync.dma_start(bd_sb[:], bd[:])
        # touch the Gelu table set up front so its ACT_TABLE_LOAD overlaps
        # the first input DMA instead of stalling the first block
        warm = const.tile([1, 1], F32)
        nc.scalar.activation(
            warm[:], bd_sb[0:1, 0:1], mybir.ActivationFunctionType.Gelu
        )

        # pair two 786 KB blocks per DMA: 1.57 MB transfers amortize the
        # fixed trigger + completion-receipt cost (~2 us each on SWDGE)
        # over 2x the bytes and sit higher on the SDMA efficiency curve;
        # the descriptor profile (6 KB contiguous per partition line) is
        # unchanged, as is all compute/PSUM structure
        x_quad = x.rearrange("(u k) p c t -> u p k c t", k=4)
        y_pair = y.rearrange("(u k) p c t -> u p k c t", k=2)
        nquad = nblk // 4

        for uo in range(nquad * 2 * repeats):
            uo = uo % (nquad * 2)
            u, half = divmod(uo, 2)
            if half == 0:
                # input batched 4 blocks (pure prefetch, no wait exposure);
                # output stays 2-block so drains release at fine grain
                xt4 = xin.tile([P, 4, HC, TBLK], BF16, tag="xin")
                nc.sync.dma_start(xt4[:], x_quad[u])
            if mode == "dmaonly":
                if half == 1:
                    continue
                ot2 = outp.tile([P, 2, HC, TBLK], BF16, tag="ot")
                nc.vector.tensor_copy(ot2[:, 0, 0, :], xt4[:, 0, 0, :])
                nc.gpsimd.dma_start(y_pair[2 * u], ot2[:])
                continue
            ot2 = outp.tile([P, 2, HC, TBLK], BF16, tag="ot")
            for k in range(2):
                xt_sb = xt4[:, half * 2 + k]
                # down projection: accumulate over hidden chunks
                pd = ps_d.tile([A + 1, TBLK], F32)
                for c in range(HC):
                    nc.tensor.matmul(
                        pd[:],
                        wd_sb[:, c, :],
                        xt_sb[:, c, :],
                        start=(c == 0),
                        stop=(c == HC - 1),
                    )
                # exact-erf gelu with per-partition b_down bias; row A
                # computes gelu(0 + gelu^-1(1)) = 1.0, the b_up multiplier
                act = actp.tile([A + 1, TBLK], BF16)
                nc.scalar.activation(
                    act[:], pd[:], mybir.ActivationFunctionType.Gelu,
                    bias=bd_sb[:, 0:1],
                )
                if mode == "front":
                    continue
                # up projection, transposed: W_up chunk stationary, act
                # moving
                for c in range(HC):
                    pu = ps_u.tile([P, TBLK], F32)
                    nc.tensor.matmul(
                        pu[:], wub_sb[:, c * P : (c + 1) * P], act[:],
                        start=True, stop=True,
                    )
                    if mode != "dveonly" and c >= HC - 2:
                        # last two chunks: drain via ACT copy + 2x SBUF add
                        # so DVE (the busiest vector engine) stays under
                        # budget
                        ub = ubp.tile([P, TBLK], BF16)
                        nc.scalar.copy(ub[:], pu[:])
                        nc.vector.tensor_add(
                            ot2[:, k, c, :], xt_sb[:, c, :], ub[:]
                        )
                    else:
                        nc.vector.tensor_add(
                            ot2[:, k, c, :], xt_sb[:, c, :], pu[:]
                        )
            if mode not in ("noout", "front"):
                nc.gpsimd.dma_start(y_pair[uo], ot2[:])
    return _legalize_waits(nc)


def _tt_to_matrix(cores, in_dim, out_dim):
    t = cores[0]
    for c in cores[1:]:
        t = np.tensordot(t, c, axes=([-1], [0]))
    t = np.squeeze(t, axis=(0, -1))
    return np.ascontiguousarray(t.reshape(in_dim, out_dim).astype(np.float32))


def _gelu_inv_one():
    """x with x * Phi(x) == 1 (erf gelu), solved by Newton in float64."""
    import math

    def gelu(x):
        return x * 0.5 * (1.0 + math.erf(x / math.sqrt(2.0)))

    def dgelu(x):
        return 0.5 * (1.0 + math.erf(x / math.sqrt(2.0))) + x * math.exp(
            -0.5 * x * x
        ) / math.sqrt(2.0 * math.pi)

    x = 1.15
    for _ in range(40):
        x -= (gelu(x) - 1.0) / dgelu(x)
    return x


_NC_CACHE = {}


def _get_nc(tok=TOK):
    if tok not in _NC_CACHE:
        _NC_CACHE[tok] = build_nc(tok)
    return _NC_CACHE[tok]


def kernel(hidden_states, d0, d1, d2, d3, d4, u0, u1, u2, u3, u4,
           b_down, b_up, **_run_kwargs):
    hs = np.asarray(hidden_states, dtype=np.float32)
    w_down = _tt_to_matrix(
        [np.asarray(c, np.float32) for c in (d0, d1, d2, d3, d4)], H, A
    )
    w_up = _tt_to_matrix(
        [np.asarray(c, np.float32) for c in (u0, u1, u2, u3, u4)], A, H
    )
    wd = np.concatenate([w_down, np.zeros((H, 1), np.float32)], axis=1)
    wd = np.ascontiguousarray(wd.astype(NPBF16))
    wub = np.ascontiguousarray(
        np.concatenate(
            [w_up, np.asarray(b_up, np.float32)[None, :]], axis=0
        ).astype(NPBF16)
    )
    bd = np.concatenate(
        [
            np.asarray(b_down, np.float32).reshape(A, 1),
            np.full((1, 1), _gelu_inv_one(), np.float32),
        ],
        axis=0,
    )
    bd = np.ascontiguousarray(bd)

    flat = hs.reshape(B * S, H).astype(NPBF16)

    def to_blocked(xc):
        # [4096, 768] -> [block, partition, chunk, token] with
        # token = b*TBLK + t, hidden = c*P + p
        return np.ascontiguousarray(
            xc.reshape(NBLK, TBLK, HC, P).transpose(0, 3, 2, 1)
        )

    in_maps = [
        {
            "x": to_blocked(flat[c * TOK : (c + 1) * TOK]),
            "wd": wd,
            "wub": wub,
            "bd": bd,
        }
        for c in range(NCORES)
    ]
    nc = _get_nc()
    res = run_bass_kernel_spmd(nc, in_maps, list(range(NCORES)), **_run_kwargs)
    out = np.concatenate(
        [
            res.results[c]["y"].transpose(0, 3, 2, 1).reshape(TOK, H)
            for c in range(NCORES)
        ],
        axis=0,
    )
    out = out.astype(np.float32).reshape(B, S, H)
    if _run_kwargs:
        kernel.last_results = res
    return out


# revision 38
# speedup vs baseline: 1.1465x; 1.0996x over previous
"""BertAdapter (TT-decomposed bottleneck MLP) Trainium2 kernel.

Computes  out = x + gelu(x @ W_down + b_down) @ W_up + b_up  where the
adapter weights arrive as tensor-train cores.  The TT cores are tiny
(~50K params), so they are contracted to dense matrices on the host and
the device kernel runs the dense bottleneck MLP data-parallel across
8 NeuronCores (2 batches of 2048 tokens per core).

The kernel is HBM-bandwidth dominated (25 MB of fp32 I/O per core vs
~20 us of PE work), so all HBM traffic is bf16 (halves the DMA floor
to ~33 us; rel-err ~5e-3 against the 2e-2 gate since PSUM accumulation
stays fp32).

The whole device pipeline runs in the TRANSPOSED domain: the host hands
each core x^T (hidden-major) and receives y^T back.  hidden-on-
partitions is exactly what the PE needs for the down-projection, so the
kernel has NO on-device transposes (an earlier natural-layout version
spent ~15 us/body of PE time on transposes plus a PSUM->SBUF drain on
the scalar engine just to feed the same matmuls).

Per-core device kernel, per 512-token block:
  1. One DMA brings in x^T [768, 512] as [128, 6, 512] bf16
     (hidden on partitions), issued from the SP sequencer (pure input
     prefetch stream; outputs leave via GPSIMD SWDGE).
  2. Down-proj: 6 accumulating bf16 matmuls -> PSUM [65, 512].
  3. Exact-erf Gelu + b_down bias on the scalar engine (bias is
     per-partition in this layout); row 64 evaluates
     gelu(gelu^-1(1)) = 1, the ones-row that multiplies the b_up row
     folded into wub.
  4. Up-proj, transposed: per hidden chunk, W_up chunk [65, 128] is the
     stationary operand and act [65, 512] moves -> PSUM [128, 512].
     PSUM pool is 6 deep, so the PE stream never waits on the drains.
  5. Residual add (x^T + up^T) on the vector engine -> bf16; chunks 4-5
     are routed ACT-copy + cheap 2x SBUF add to keep DVE off the
     critical path.  One DMA out (GPSIMD).
"""

import os
import sys
from contextlib import ExitStack

import numpy as np
import ml_dtypes

for _p in ("/opt/trn_rl_repo", "/root/.axon_site/_ro/trn_rl_repo"):
    if os.path.isdir(_p) and _p not in sys.path:
        sys.path.insert(0, _p)

import concourse.bass as bass
import concourse.tile as tile
from concourse import mybir
from concourse.bass_utils import run_bass_kernel_spmd

P = 128                 # SBUF partitions
H = 768                 # hidden size
A = 64                  # adapter bottleneck size
B, S = 16, 2048         # full batch / seq
NCORES = 8
TOK = (B // NCORES) * S  # tokens per core = 4096
TBLK = 512              # tokens per pipeline block
NBLK = TOK // TBLK
HC = H // P             # hidden chunks of 128
F32 = mybir.dt.float32
BF16 = mybir.dt.bfloat16
NPBF16 = ml_dtypes.bfloat16
X_SHAPE = (NBLK, P, HC, TBLK)  # per-core transposed + blocked device I/O


_TileContext = tile.TileContext


def _legalize_waits(nc):
    """Split multi-wait instructions for this walrus build.

    The walrus in this toolchain accepts only ONE sync-wait per
    instruction ("Too many sync wait commands" in setupSyncWait), while
    Tile freely attaches several.  Hoist all but the last wait of each
    instruction onto freshly inserted same-engine NoOps directly before
    it — engine program order makes this semantically identical.
    """
    n = 0

    def fix_block(bb):
        nonlocal n
        insts = bb.instructions
        i = 0
        while i < len(insts):
            inst = insts[i]
            for sub in getattr(inst, "blocks", None) or []:
                fix_block(sub)
            si = inst.sync_info
            waits = list(si.on_wait) if si and si.on_wait else []
            if len(waits) > 1:
                for w in waits[:-1]:
                    nop = mybir.InstNoOp(name=f"I-waitsplit-{n}", ins=[], outs=[])
                    n += 1
                    nop.engine = inst.engine
                    nop.sync_info = mybir.SyncInfo(on_wait=[w], on_update=[])
                    insts.insert(i, nop)
                    i += 1
                inst.sync_info = mybir.SyncInfo(
                    on_wait=[waits[-1]], on_update=list(si.on_update)
                )
            i += 1

    for fn in nc.m.functions:
        for bb in fn.blocks:
            fix_block(bb)
    return nc


def build_nc(tok=TOK, repeats=1, mode="full"):
    nblk = tok // TBLK
    nc = bass.Bass("TRN2", target_bir_lowering=False, debug=False)
    # x/y are the per-core shard transposed AND pre-blocked on the host to
    # [block, partition, chunk, token]: every per-block DMA is one fully
    # contiguous 786 KB read/write with 6 KB per partition line
    x = nc.dram_tensor(
        "x", [nblk, P, HC, TBLK], BF16, kind="ExternalInput"
    ).ap()
    # wd/bd carry an extra adapter column: wd col A is zeros and bd[A] is
    # gelu^-1(1.0), so the gelu writes a constant ones-row into act[A] that
    # multiplies the b_up row of wub in the up-projection (bias via matmul).
    wd = nc.dram_tensor("wd", [H, A + 1], BF16, kind="ExternalInput").ap()
    wub = nc.dram_tensor("wub", [A + 1, H], BF16, kind="ExternalInput").ap()
    bd = nc.dram_tensor("bd", [A + 1, 1], F32, kind="ExternalInput").ap()
    y = nc.dram_tensor(
        "y", [nblk, P, HC, TBLK], BF16, kind="ExternalOutput"
    ).ap()

    with ExitStack() as ctx:
        tc = ctx.enter_context(_TileContext(nc))
        const = ctx.enter_context(tc.tile_pool(name="const", bufs=1))
        xin = ctx.enter_context(tc.tile_pool(name="xin", bufs=4))
        actp = ctx.enter_context(tc.tile_pool(name="act", bufs=2))
        outp = ctx.enter_context(tc.tile_pool(name="out", bufs=4))
        ubp = ctx.enter_context(tc.tile_pool(name="ub", bufs=3))
        ps_d = ctx.enter_context(tc.tile_pool(name="ps_d", bufs=2, space="PSUM"))
        ps_u = ctx.enter_context(tc.tile_pool(name="ps_u", bufs=6, space="PSUM"))

        wd_sb = const.tile([P, HC, A + 1], BF16)
        nc.sync.dma_start(wd_sb[:], wd.rearrange("(c p) a -> p c a", p=P))
        wub_sb = const.tile([A + 1, H], BF16)
        nc.sync.dma_start(wub_sb[:], wub[:])
        bd_sb = const.tile([A + 1, 1], F32)
        nc.sync.dma_start(bd_sb[:], bd[:])
        # touch the Gelu table set up front so its ACT_TABLE_LOAD overlaps
        # the first input DMA instead of stalling the first block
        warm = const.tile([1, 1], F32)
        nc.scalar.activation(
            warm[:], bd_sb[0:1, 0:1], mybir.ActivationFunctionType.Gelu
        )

        # pair two 786 KB blocks per DMA: 1.57 MB transfers amortize the
        # fixed trigger + completion-receipt cost (~2 us each on SWDGE)
        # over 2x the bytes and sit higher on the SDMA efficiency curve;
        # the descriptor profile (6 KB contiguous per partition line) is
        # unchanged, as is all compute/PSUM structure
        x_pair = x.rearrange("(u k) p c t -> u p k c t", k=2)
        y_pair = y.rearrange("(u k) p c t -> u p k c t", k=2)
        npair = nblk // 2

        for u in range(npair * repeats):
            u = u % npair
            xt2 = xin.tile([P, 2, HC, TBLK], BF16, tag="xin")
            nc.sync.dma_start(xt2[:], x_pair[u])
            if mode == "dmaonly":
                ot2 = outp.tile([P, 2, HC, TBLK], BF16, tag="ot")
                nc.vector.tensor_copy(ot2[:, 0, 0, :], xt2[:, 0, 0, :])
                nc.gpsimd.dma_start(y_pair[u], ot2[:])
                continue
            ot2 = outp.tile([P, 2, HC, TBLK], BF16, tag="ot")
            for k in range(2):
                xt_sb = xt2[:, k]
                # down projection: accumulate over hidden chunks
                pd = ps_d.tile([A + 1, TBLK], F32)
                for c in range(HC):
                    nc.tensor.matmul(
                        pd[:],
                        wd_sb[:, c, :],
                        xt_sb[:, c, :],
                        start=(c == 0),
                        stop=(c == HC - 1),
                    )
                # exact-erf gelu with per-partition b_down bias; row A
                # computes gelu(0 + gelu^-1(1)) = 1.0, the b_up multiplier
                act = actp.tile([A + 1, TBLK], BF16)
                nc.scalar.activation(
                    act[:], pd[:], mybir.ActivationFunctionType.Gelu,
                    bias=bd_sb[:, 0:1],
                )
                if mode == "front":
                    continue
                # up projection, transposed: W_up chunk stationary, act
                # moving
                for c in range(HC):
                    pu = ps_u.tile([P, TBLK], F32)
                    nc.tensor.matmul(
                        pu[:], wub_sb[:, c * P : (c + 1) * P], act[:],
                        start=True, stop=True,
                    )
                    if mode != "dveonly" and c >= HC - 2:
                        # last two chunks: drain via ACT copy + 2x SBUF add
                        # so DVE (the busiest vector engine) stays under
                        # budget
                        ub = ubp.tile([P, TBLK], BF16)
                        nc.scalar.copy(ub[:], pu[:])
                        nc.vector.tensor_add(
                            ot2[:, k, c, :], xt_sb[:, c, :], ub[:]
                        )
                    else:
                        nc.vector.tensor_add(
                            ot2[:, k, c, :], xt_sb[:, c, :], pu[:]
                        )
            if mode not in ("noout", "front"):
                if mode == "outgp":
                    nc.gpsimd.dma_start(y_pair[u], ot2[:])
                else:
                    # ACT has ~2.3us/block slack in this dataflow, so its
                    # HWDGE ring (hw-built descriptors, ~0.6us fixed) beats
                    # the GPSIMD SWDGE path (~1-2us software build)
                    nc.scalar.dma_start(y_pair[u], ot2[:])
    return _legalize_waits(nc)


def _tt_to_matrix(cores, in_dim, out_dim):
    t = cores[0]
    for c in cores[1:]:
        t = np.tensordot(t, c, axes=([-1], [0]))
    t = np.squeeze(t, axis=(0, -1))
    return np.ascontiguousarray(t.reshape(in_dim, out_dim).astype(np.float32))


def _gelu_inv_one():
    """x with x * Phi(x) == 1 (erf gelu), solved by Newton in float64."""
    import math

    def gelu(x):
        return x * 0.5 * (1.0 + math.erf(x / math.sqrt(2.0)))

    def dgelu(x):
        return 0.5 * (1.0 + math.erf(x / math.sqrt(2.0))) + x * math.exp(
            -0.5 * x * x
        ) / math.sqrt(2.0 * math.pi)

    x = 1.15
    for _ in range(40):
        x -= (gelu(x) - 1.0) / dgelu(x)
    return x


_NC_CACHE = {}


def _get_nc(tok=TOK):
    if tok not in _NC_CACHE:
        _NC_CACHE[tok] = build_nc(tok)
    return _NC_CACHE[tok]


def kernel(hidden_states, d0, d1, d2, d3, d4, u0, u1, u2, u3, u4,
           b_down, b_up, **_run_kwargs):
    hs = np.asarray(hidden_states, dtype=np.float32)
    w_down = _tt_to_matrix(
        [np.asarray(c, np.float32) for c in (d0, d1, d2, d3, d4)], H, A
    )
    w_up = _tt_to_matrix(
        [np.asarray(c, np.float32) for c in (u0, u1, u2, u3, u4)], A, H
    )
    wd = np.concatenate([w_down, np.zeros((H, 1), np.float32)], axis=1)
    wd = np.ascontiguousarray(wd.astype(NPBF16))
    wub = np.ascontiguousarray(
        np.concatenate(
            [w_up, np.asarray(b_up, np.float32)[None, :]], axis=0
        ).astype(NPBF16)
    )
    bd = np.concatenate(
        [
            np.asarray(b_down, np.float32).reshape(A, 1),
            np.full((1, 1), _gelu_inv_one(), np.float32),
        ],
        axis=0,
    )
    bd = np.ascontiguousarray(bd)

    flat = hs.reshape(B * S, H).astype(NPBF16)

    def to_blocked(xc):
        # [4096, 768] -> [block, partition, chunk, token] with
        # token = b*TBLK + t, hidden = c*P + p
        return np.ascontiguousarray(
            xc.reshape(NBLK, TBLK, HC, P).transpose(0, 3, 2, 1)
        )

    in_maps = [
        {
            "x": to_blocked(flat[c * TOK : (c + 1) * TOK]),
            "wd": wd,
            "wub": wub,
            "bd": bd,
        }
        for c in range(NCORES)
    ]
    nc = _get_nc()
    res = run_bass_kernel_spmd(nc, in_maps, list(range(NCORES)), **_run_kwargs)
    out = np.concatenate(
        [
            res.results[c]["y"].transpose(0, 3, 2, 1).reshape(TOK, H)
            for c in range(NCORES)
        ],
        axis=0,
    )
    out = out.astype(np.float32).reshape(B, S, H)
    if _run_kwargs:
        kernel.last_results = res
    return out
